# revision 10
# baseline (speedup 1.0000x reference)
"""Gated-relative-position-bias multi-head attention, 8-way tensor-parallel
over heads on Trainium2 (Bass/Tile).

Shapes: x (2, 2048, 1024), 16 heads x 64 head-dim, position_bias
(16, 2048, 2048), per-query sigmoid gates computed from x.

Sharding: core c owns heads (2c, 2c+1) = feature slice [128c, 128c+128).
Each core computes q/k/v for its heads, the gated-bias attention, and a
partial output projection (O_g @ Wo_g.T).  The host sums the 8 partials and
adds bo.

Per-core strategy:
  - host passes xT (x transposed, [D, B*T]) so the d-contraction sits on
    partitions; qT/kT/vT are computed weights-stationary (1/sqrt(hd) folded
    into Wq/bq on the host).
  - scores are computed TRANSPOSED: sT[k, q] = kT.T @ qT (K=hd=64), so the
    attn @ v contraction (over k) has k on partitions.
  - the gated position bias is added into the scores PSUM by the PE via an
    identity matmul (psum += I.T @ pbg); pbg = pbT * gate_bcast is formed on
    the DVE in bf16 (2x mode), gate_bcast via gpsimd partition_broadcast.
  - softmax needs no max-subtraction (scores are O(+-3) for this model
    family); denominators come free as an all-ones column of v_aug (row 64
    of the AV matmul PSUM output).
  - normalization happens during the PSUM->SBUF evacuation of O; the
    output projection then runs on the stacked normalized O^T.
"""

import sys

sys.path.insert(0, "/opt/trn_rl_repo")

import ml_dtypes
import numpy as np

import concourse.mybir as mybir
import concourse.tile as tile
from concourse import bacc
from concourse.bass_utils import run_bass_kernel_spmd

F32 = mybir.dt.float32
BF16 = mybir.dt.bfloat16
AF = mybir.ActivationFunctionType
ALU = mybir.AluOpType

B, T, D, H, HD = 2, 2048, 1024, 16, 64
NCORES = 8
HPC = H // NCORES          # heads per core = 2
FPC = HPC * HD             # features per core = 128
BT = B * T                 # 4096
P = 128
NKC = T // P               # key chunks = 16
NQH = 2                    # q halves per (h, b) block
QH = T // NQH              # 1024

# test.py hooks
TRACE = False
LAST_RESULT = None


def _build_program():
    nc = bacc.Bacc("TRN2", target_bir_lowering=False, debug=False,
                   num_devices=NCORES)

    xT = nc.dram_tensor("xT", [D, BT], BF16, kind="ExternalInput")
    xg = nc.dram_tensor("xg", [P, BT], BF16, kind="ExternalInput")
    wq = nc.dram_tensor("wq", [D, FPC], BF16, kind="ExternalInput")
    wk = nc.dram_tensor("wk", [D, FPC], BF16, kind="ExternalInput")
    wv = nc.dram_tensor("wv", [D, FPC], BF16, kind="ExternalInput")
    bq = nc.dram_tensor("bq", [FPC], F32, kind="ExternalInput")
    bk = nc.dram_tensor("bk", [FPC], F32, kind="ExternalInput")
    bv = nc.dram_tensor("bv", [FPC], F32, kind="ExternalInput")
    wo = nc.dram_tensor("wo", [FPC, D], BF16, kind="ExternalInput")
    pbt = nc.dram_tensor("pbt", [HPC, T, T], BF16, kind="ExternalInput")
    wg2 = nc.dram_tensor("wg2", [P, 97], BF16, kind="ExternalInput")
    bg2 = nc.dram_tensor("bg2", [97], F32, kind="ExternalInput")
    gc2 = nc.dram_tensor("gc2", [97], F32, kind="ExternalInput")
    id32 = nc.dram_tensor("id32", [P, P], F32, kind="ExternalInput")
    idb = nc.dram_tensor("idb", [P, P], BF16, kind="ExternalInput")
    out = nc.dram_tensor("out", [BT, D], F32, kind="ExternalOutput")

    with tile.TileContext(nc) as tc:
        with tc.tile_pool(name="const", bufs=1) as const, \
             tc.tile_pool(name="big", bufs=1) as big:
            id32_t = const.tile([P, P], F32, tag="id32")
            nc.sync.dma_start(id32_t[:], id32[:])
            ones_t = const.tile([P, HD], F32, tag="ones")
            nc.vector.memset(ones_t[:], 1.0)
            idb_t = const.tile([P, P], BF16, tag="idb")
            nc.sync.dma_start(idb_t[:], idb[:])
            w_ts = {}
            for name, dram in (("wq", wq), ("wk", wk), ("wv", wv)):
                w_t = const.tile([P, D // P, FPC], BF16, tag=name)
                nc.sync.dma_start(
                    w_t[:], dram.rearrange("(c p) f -> p c f", p=P))
                w_ts[name] = w_t
            b_ts = {}
            for name, dram in (("bq", bq), ("bk", bk), ("bv", bv)):
                b_t = const.tile([FPC, 1], F32, tag=name)
                nc.sync.dma_start(b_t[:], dram.rearrange("(p o) -> p o", o=1))
                b_ts[name] = b_t
            wo_t = const.tile([FPC, D], BF16, tag="wo")
            nc.sync.dma_start(wo_t[:], wo[:])
            wg2_t = const.tile([P, 97], BF16, tag="wg2")
            nc.sync.dma_start(wg2_t[:], wg2[:])
            bg2_t = const.tile([97, 1], F32, tag="bg2")
            nc.sync.dma_start(bg2_t[:], bg2.rearrange("(p o) -> p o", o=1))
            gc_t = const.tile([97, 1], F32, tag="gc")
            nc.sync.dma_start(gc_t[:], gc2.rearrange("(p o) -> p o", o=1))

            qT = big.tile([FPC, BT], BF16, tag="qT")
            kT = big.tile([FPC, BT], BF16, tag="kT")
            # gate_input per head, bf16, one row each (partition base 0)
            G2h = [big.tile([1, BT], BF16, tag=f"G2h{h}", name=f"G2h{h}")
                   for h in range(HPC)]
            OT = [big.tile([FPC, T], BF16, tag=f"OT{b}", name=f"OT{b}")
                  for b in range(B)]
            vaug = {(h, b): big.tile([P, NKC * (HD + 1)], BF16,
                                     tag=f"va{h}{b}", name=f"va{h}{b}")
                    for h in range(HPC) for b in range(B)}

            # ---------------- Phase 1: q/k/v projections + gate ----------
            with tc.tile_pool(name="xt", bufs=1) as xt_pool, \
                 tc.tile_pool(name="xgp", bufs=2) as xg_pool, \
                 tc.tile_pool(name="vt", bufs=1) as vt_pool, \
                 tc.tile_pool(name="gp", bufs=1) as g_pool, \
                 tc.tile_pool(name="gtmp", bufs=1) as gtmp_pool, \
                 tc.tile_pool(name="ps_qkv", bufs=3, space="PSUM") as ps_qkv, \
                 tc.tile_pool(name="ps_g", bufs=2, space="PSUM") as ps_g:
                vT = vt_pool.tile([FPC, BT], BF16, tag="vT")
                # gate rows at 32-aligned partitions:
                # row 0 = h0 gate-a, 32 = h1 gate-a, 64 = h0 gate-b,
                # 96 = h1 gate-b
                G = g_pool.tile([97, BT], BF16, tag="G")
                for th in range(4):
                    tsl = slice(th * (BT // 4), (th + 1) * (BT // 4))
                    xts = []
                    for d in range(D // P):
                        xt_t = xt_pool.tile([P, BT // 4], BF16, tag=f"xt{d}")
                        nc.sync.dma_start(
                            xt_t[:], xT[d * P:(d + 1) * P, tsl])
                        xts.append(xt_t)
                    for ti in range(BT // 4 // 512):
                        c0 = th * (BT // 4) + ti * 512
                        for wname, bname, dst in (("wq", "bq", qT),
                                                  ("wk", "bk", kT),
                                                  ("wv", "bv", vT)):
                            ps = ps_qkv.tile([FPC, 512], F32, tag="ps")
                            for d in range(D // P):
                                nc.tensor.matmul(
                                    ps[:], w_ts[wname][:, d, :],
                                    xts[d][:, ti * 512:(ti + 1) * 512],
                                    start=(d == 0), stop=(d == D // P - 1))
                            nc.vector.tensor_scalar(
                                out=dst[:, c0:c0 + 512], in0=ps[:],
                                scalar1=b_ts[bname][:], scalar2=None,
                                op0=ALU.add)
                        xg_t = xg_pool.tile([P, 512], BF16, tag="xgc")
                        nc.sync.dma_start(xg_t[:], xg[:, c0:c0 + 512])
                        psg = ps_g.tile([97, 512], F32, tag="psg")
                        nc.tensor.matmul(psg[:], wg2_t[:], xg_t[:],
                                         start=True, stop=True)
                        nc.scalar.activation(G[:, c0:c0 + 512], psg[:],
                                             AF.Sigmoid, bias=bg2_t[:])

                # gate combine per head: G2 = a * (b * gc - 1) + 2
                gt1 = gtmp_pool.tile([33, BT], F32, tag="gt1")
                for h in range(HPC):
                    r = 32 * h
                    nc.vector.tensor_scalar(
                        out=gt1[r:r + 1, :], in0=G[64 + r:65 + r, :],
                        scalar1=gc_t[64 + r:65 + r, :], scalar2=-1.0,
                        op0=ALU.mult, op1=ALU.add)
                    nc.vector.tensor_mul(G2h[h][:],
                                         G[r:r + 1, :], gt1[r:r + 1, :])
                    nc.vector.tensor_scalar(
                        out=G2h[h][:], in0=G2h[h][:], scalar1=2.0,
                        scalar2=None, op0=ALU.add)

                # v_aug: transposed v chunks + an all-ones column per chunk
                for h in range(HPC):
                    hsl = slice(h * HD, (h + 1) * HD)
                    for b in range(B):
                        va = vaug[(h, b)]
                        nc.vector.memset(va[:], 1.0)
                        for kc in range(NKC):
                            pst = ps_g.tile([P, HD], BF16, tag="pst")
                            nc.tensor.transpose(
                                pst[:],
                                vT[hsl, b * T + kc * P: b * T + (kc + 1) * P],
                                idb_t[hsl, hsl])
                            nc.any.tensor_copy(
                                va[:, kc * (HD + 1):kc * (HD + 1) + HD],
                                pst[:])

            # ---------------- Phase 2: attention -------------------------
            with tc.tile_pool(name="pb", bufs=3) as pb_pool, \
                 tc.tile_pool(name="pbg", bufs=3) as pbg_pool, \
                 tc.tile_pool(name="expp", bufs=4) as exp_pool, \
                 tc.tile_pool(name="gbc", bufs=1) as gbc_pool, \
                 tc.tile_pool(name="rbp", bufs=4) as rb_pool, \
                 tc.tile_pool(name="osb", bufs=3) as osb_pool, \
                 tc.tile_pool(name="ps_sc", bufs=3, space="PSUM") as ps_sc, \
                 tc.tile_pool(name="ps_av", bufs=5, space="PSUM") as ps_av:
                # prefetch all gate broadcasts (gpsimd, off critical path)
                gbcs = {}
                for h in range(HPC):
                    for b in range(B):
                        gbc = gbc_pool.tile([P, T], BF16, tag=f"gbc{h}{b}",
                                            name=f"gbc{h}{b}")
                        nc.gpsimd.partition_broadcast(
                            gbc[:], G2h[h][0:1, b * T:(b + 1) * T])
                        gbcs[(h, b)] = gbc
                NQQ = T // 512          # q quarters per (h, b) block
                for b in range(B):
                    for h in range(HPC):
                        hsl = slice(h * HD, (h + 1) * HD)
                        va = vaug[(h, b)]
                        avs = [ps_av.tile([HD + 1, 512], F32, tag="av",
                                          name=f"av{h}{b}{qq}")
                               for qq in range(NQQ)]
                        for kc in range(NKC):
                            pbt_t = pb_pool.tile([P, T], BF16, tag="pb")
                            nc.sync.dma_start(
                                pbt_t[:], pbt[h, kc * P:(kc + 1) * P, :])
                            pbg = pbg_pool.tile([P, T], BF16, tag="pbg")
                            eng = nc.gpsimd if kc % 2 == 0 else nc.vector
                            eng.tensor_tensor(out=pbg[:], in0=pbt_t[:],
                                              in1=gbcs[(h, b)][:],
                                              op=ALU.mult)
                            lk = kT[hsl, b * T + kc * P: b * T + (kc + 1) * P]
                            vak = va[:, kc * (HD + 1):(kc + 1) * (HD + 1)]
                            for qq in range(NQQ):
                                q0 = b * T + qq * 512
                                sc = ps_sc.tile([P, 512], F32, tag="sc")
                                nc.tensor.matmul(sc[:], lk,
                                                 qT[hsl, q0:q0 + 512],
                                                 start=True, stop=False)
                                nc.tensor.matmul(
                                    sc[:], idb_t[:],
                                    pbg[:, qq * 512:(qq + 1) * 512],
                                    start=False, stop=True)
                                ex = exp_pool.tile([P, 512], BF16, tag="ex")
                                nc.scalar.activation(ex[:], sc[:], AF.Exp)
                                nc.tensor.matmul(
                                    avs[qq][:], vak, ex[:],
                                    start=(kc == 0), stop=(kc == NKC - 1))
                        for qq in range(NQQ):
                            # denominators live in row HD of avs[qq]
                            dst = rb_pool.tile([HD + 1, 512], F32, tag="dst")
                            nc.any.tensor_copy(dst[HD:HD + 1, :],
                                               avs[qq][HD:HD + 1, :])
                            rbp = ps_sc.tile([HD, 512], F32, tag="sc",
                                             name=f"rbp{h}{b}{qq}")
                            nc.tensor.matmul(rbp[:], ones_t[HD:HD + 1, :],
                                             dst[HD:HD + 1, :],
                                             start=True, stop=True)
                            rbr = rb_pool.tile([HD, 512], F32, tag="rbr")
                            nc.vector.reciprocal_approx_fast(rbr[:], rbp[:])
                            nc.vector.tensor_mul(
                                OT[b][hsl, qq * 512:(qq + 1) * 512],
                                avs[qq][0:HD, :], rbr[:])
                    # output projection for this batch overlaps the next
                    # batch's attention blocks
                    for tt in range(T // P):
                        for s in range(D // 512):
                            po = ps_sc.tile([P, 512], F32, tag="sc",
                                            name=f"po{b}{tt}{s}")
                            nc.tensor.matmul(
                                po[:],
                                OT[b][:, tt * P:(tt + 1) * P],
                                wo_t[:, s * 512:(s + 1) * 512],
                                start=True, stop=True)
                            ob = osb_pool.tile([P, 512], F32, tag="ob")
                            nc.vector.tensor_copy(ob[:], po[:])
                            nc.sync.dma_start(
                                out[b * T + tt * P: b * T + (tt + 1) * P,
                                    s * 512:(s + 1) * 512],
                                ob[:])

    nc.compile()
    return nc


_PROGRAM = None


def _get_program():
    global _PROGRAM
    if _PROGRAM is None:
        _PROGRAM = _build_program()
    return _PROGRAM


def kernel(x, position_bias, Wq, bq, Wk, bk, Wv, bv, Wo, bo, Wg, bg,
           gru_const):
    global LAST_RESULT
    x = np.asarray(x, dtype=np.float32)
    position_bias = np.asarray(position_bias, dtype=np.float32)
    Wq = np.asarray(Wq, dtype=np.float32)
    Wk = np.asarray(Wk, dtype=np.float32)
    Wv = np.asarray(Wv, dtype=np.float32)
    Wo = np.asarray(Wo, dtype=np.float32)
    bq = np.asarray(bq, dtype=np.float32)
    bk = np.asarray(bk, dtype=np.float32)
    bv = np.asarray(bv, dtype=np.float32)
    bo = np.asarray(bo, dtype=np.float32)
    Wg = np.asarray(Wg, dtype=np.float32)
    bg = np.asarray(bg, dtype=np.float32)
    gru_const = np.asarray(gru_const, dtype=np.float32)

    scale = np.float32(1.0 / np.sqrt(np.float32(HD)))

    xT_np = np.ascontiguousarray(x.reshape(BT, D).T)           # [D, BT]
    id32_np = np.eye(P, dtype=np.float32)
    idb_np = np.eye(P).astype(ml_dtypes.bfloat16)
    # the reshape-(2,4)-sum of the 8 gate features is linear -> fold into
    # the weights:  Wg2[g] = sum of Wg rows [4g, 4g+4)
    Wg2 = Wg.reshape(2, 4, HD).sum(1)                          # [2, HD]
    bg2v = bg.reshape(2, 4).sum(1)                             # [2]

    in_maps = []
    for c in range(NCORES):
        fsl = slice(c * FPC, (c + 1) * FPC)
        wg2_np = np.zeros((P, 97), dtype=np.float32)
        bg2_np = np.zeros((97,), dtype=np.float32)
        # rows 0/32 = gate-a for head0/head1; rows 64/96 = gate-b
        wg2_np[0:HD, 0] = Wg2[0]
        wg2_np[HD:P, 32] = Wg2[0]
        wg2_np[0:HD, 64] = Wg2[1]
        wg2_np[HD:P, 96] = Wg2[1]
        bg2_np[[0, 32]] = bg2v[0]
        bg2_np[[64, 96]] = bg2v[1]
        gc2_np = np.zeros((97,), dtype=np.float32)
        gc2_np[64] = gru_const[0, c * HPC, 0, 0]
        gc2_np[96] = gru_const[0, c * HPC + 1, 0, 0]
        in_maps.append({
            "xT": xT_np.astype(ml_dtypes.bfloat16),
            "xg": np.ascontiguousarray(xT_np[fsl, :]).astype(ml_dtypes.bfloat16),
            "wq": (np.ascontiguousarray(Wq.T[:, fsl]) * scale).astype(ml_dtypes.bfloat16),
            "wk": np.ascontiguousarray(Wk.T[:, fsl]).astype(ml_dtypes.bfloat16),
            "wv": np.ascontiguousarray(Wv.T[:, fsl]).astype(ml_dtypes.bfloat16),
            "bq": np.ascontiguousarray(bq[fsl]) * scale,
            "bk": np.ascontiguousarray(bk[fsl]),
            "bv": np.ascontiguousarray(bv[fsl]),
            "wo": np.ascontiguousarray(Wo[:, fsl].T).astype(ml_dtypes.bfloat16),
            "pbt": np.ascontiguousarray(
                position_bias[c * HPC:(c + 1) * HPC].transpose(0, 2, 1)
            ).astype(ml_dtypes.bfloat16),
            "wg2": wg2_np.astype(ml_dtypes.bfloat16),
            "bg2": bg2_np,
            "gc2": gc2_np,
            "id32": id32_np,
            "idb": idb_np,
        })

    nc = _get_program()
    res = run_bass_kernel_spmd(nc, in_maps, core_ids=list(range(NCORES)),
                               trace=TRACE)
    LAST_RESULT = res
    acc = res.results[0]["out"].astype(np.float32).copy()
    for c in range(1, NCORES):
        acc += res.results[c]["out"]
    acc += bo[None, :]
    return acc.reshape(B, T, D)
